# revision 1
# baseline (speedup 1.0000x reference)
"""Reconstruction of the first working kernel (rel err 2.3e-4)."""

import os
from contextlib import ExitStack
from functools import lru_cache

import numpy as np

B = 2
N = 2048
D_MODEL = 1024
N_HEADS = 16
D_HEAD = 64
T_WAKE = 0.315
OMEGA_LANG = 1.0 - T_WAKE
OMEGA_GRAV = T_WAKE
C1 = OMEGA_LANG / np.sqrt(D_HEAD)
C2 = OMEGA_GRAV
CB = -OMEGA_GRAV / 2.0

NCORES = 8
MC = 2 * D_HEAD
NTOK = B * N
CHUNK = 512
NCHUNK = NTOK // CHUNK
JT = 128
NJT = NTOK // JT
NJT_B = N // JT
DT = D_MODEL // 128
DH = D_HEAD

last_results = None


@lru_cache(maxsize=1)
def _build():
    import concourse.bacc as bacc
    import concourse.mybir as mybir
    import concourse.tile as tile
    from concourse.masks import make_identity

    f32 = mybir.dt.float32
    f32r = mybir.dt.float32r
    Exp = mybir.ActivationFunctionType.Exp

    nc = bacc.Bacc(None, target_bir_lowering=False, debug=False)
    xt = nc.dram_tensor("xt", [D_MODEL, NTOK], f32r, kind="ExternalInput")
    wp = nc.dram_tensor("wp", [D_MODEL, MC], f32r, kind="ExternalInput")
    wk = nc.dram_tensor("wk", [D_MODEL, MC], f32r, kind="ExternalInput")
    wv = nc.dram_tensor("wv", [D_MODEL, MC], f32r, kind="ExternalInput")
    wo = nc.dram_tensor("wo", [MC, D_MODEL], f32r, kind="ExternalInput")
    yt = nc.dram_tensor("yt", [D_MODEL, NTOK], f32, kind="ExternalOutput")

    with ExitStack() as ctx:
        tc = ctx.enter_context(tile.TileContext(nc))

        const = ctx.enter_context(tc.tile_pool(name="const", bufs=1))
        persist = ctx.enter_context(tc.tile_pool(name="persist", bufs=1))
        xpool = ctx.enter_context(tc.tile_pool(name="xpool", bufs=3))
        tmp1 = ctx.enter_context(tc.tile_pool(name="tmp1", bufs=2))
        epool = ctx.enter_context(tc.tile_pool(name="epool", bufs=6))
        opool = ctx.enter_context(tc.tile_pool(name="opool", bufs=2))
        ypool = ctx.enter_context(tc.tile_pool(name="ypool", bufs=2))
        ps_proj = ctx.enter_context(tc.tile_pool(name="ps_proj", bufs=1, space="PSUM"))
        ps_trsq = ctx.enter_context(tc.tile_pool(name="ps_trsq", bufs=1, space="PSUM"))
        ps_s = ctx.enter_context(tc.tile_pool(name="ps_s", bufs=2, space="PSUM"))
        ps_o = ctx.enter_context(tc.tile_pool(name="ps_o", bufs=1, space="PSUM"))

        wp_sb = const.tile([128, DT, MC], f32r, tag="wp")
        wk_sb = const.tile([128, DT, MC], f32r, tag="wk")
        wv_sb = const.tile([128, DT, MC], f32r, tag="wv")
        wo_sb = const.tile([MC, DT, 128], f32r, tag="wo")
        nc.sync.dma_start(out=wp_sb, in_=wp.rearrange("(a p) m -> p a m", p=128))
        nc.sync.dma_start(out=wk_sb, in_=wk.rearrange("(a p) m -> p a m", p=128))
        nc.sync.dma_start(out=wv_sb, in_=wv.rearrange("(a p) m -> p a m", p=128))
        nc.sync.dma_start(out=wo_sb, in_=wo.rearrange("d (a o) -> d a o", o=128))

        ident = const.tile([128, 128], f32, tag="ident")
        make_identity(nc, ident)
        onescb = const.tile([128, 2], f32, tag="onescb")
        nc.vector.memset(onescb, CB)

        pt_sb = persist.tile([128, NTOK], f32r, tag="pt")
        kt_sb = persist.tile([128, NTOK], f32r, tag="kt")
        v_sb = persist.tile([128, NJT, 2 * (DH + 1)], f32r, tag="v")
        eb_sb = persist.tile([128, 2 * NJT], f32, tag="eb")

        for t0 in range(NCHUNK):
            tsl = slice(t0 * CHUNK, (t0 + 1) * CHUNK)
            xt_sb = xpool.tile([128, DT, CHUNK], f32r, tag="xt")
            nc.sync.dma_start(
                out=xt_sb, in_=xt.rearrange("(a p) t -> p a t", p=128)[:, :, tsl]
            )

            pk = ps_proj.tile([128, CHUNK], f32, tag="proj")
            for a in range(DT):
                nc.tensor.matmul(
                    pk, lhsT=wk_sb[:, a, :], rhs=xt_sb[:, a, :],
                    start=(a == 0), stop=(a == DT - 1),
                )
            nc.scalar.copy(kt_sb[:, tsl], pk)
            ksq = tmp1.tile([128, CHUNK], f32, tag="ksq")
            nc.scalar.square(ksq, pk)

            for sub in range(CHUNK // JT):
                jt = t0 * (CHUNK // JT) + sub
                jsl = slice(sub * JT, (sub + 1) * JT)
                for h in range(2):
                    psq = ps_trsq.tile([128, 512], f32, tag="trsq")
                    nc.tensor.matmul(
                        psq[:, 0:2],
                        lhsT=ksq[h * 64 : (h + 1) * 64, jsl],
                        rhs=onescb[h * 64 : (h + 1) * 64, :],
                        start=True, stop=True,
                    )
                    nc.scalar.activation(
                        eb_sb[:, 2 * jt + h : 2 * jt + h + 1], psq[:, 0:1], Exp
                    )

            pp = ps_proj.tile([128, CHUNK], f32, tag="proj")
            for a in range(DT):
                nc.tensor.matmul(
                    pp, lhsT=wp_sb[:, a, :], rhs=xt_sb[:, a, :],
                    start=(a == 0), stop=(a == DT - 1),
                )
            nc.scalar.copy(pt_sb[:, tsl], pp)

            pv = ps_proj.tile([128, CHUNK], f32, tag="proj")
            for a in range(DT):
                nc.tensor.matmul(
                    pv, lhsT=wv_sb[:, a, :], rhs=xt_sb[:, a, :],
                    start=(a == 0), stop=(a == DT - 1),
                )
            vt_sb = tmp1.tile([128, CHUNK], f32, tag="vt")
            nc.vector.tensor_copy(vt_sb, pv)
            for sub in range(CHUNK // JT):
                jt = t0 * (CHUNK // JT) + sub
                ptr = ps_trsq.tile([128, 512], f32, tag="trsq")
                nc.tensor.transpose(
                    ptr[:, 0:128], vt_sb[:, sub * JT : (sub + 1) * JT], ident
                )
                c0 = 2 * jt
                nc.vector.tensor_scalar_mul(
                    v_sb[:, jt, 0:DH], ptr[:, 0:64], eb_sb[:, c0 : c0 + 1]
                )
                nc.vector.tensor_copy(v_sb[:, jt, DH : DH + 1], eb_sb[:, c0 : c0 + 1])
                nc.vector.tensor_scalar_mul(
                    v_sb[:, jt, DH + 1 : 2 * DH + 1], ptr[:, 64:128],
                    eb_sb[:, c0 + 1 : c0 + 2],
                )
                nc.vector.tensor_copy(
                    v_sb[:, jt, 2 * DH + 1 : 2 * DH + 2], eb_sb[:, c0 + 1 : c0 + 2]
                )

        def y_step(ohn, ot):
            py = ps_s.tile([128, CHUNK], f32, tag="sy", name="py")
            nc.tensor.matmul(
                py, lhsT=wo_sb[:, ot, :], rhs=ohn, start=True, stop=True
            )
            y_sb = y_state["y_sb"]
            nc.vector.tensor_copy(y_sb[:, ot, :], py)

        y_state = {"y_sb": None}
        pending = None
        for b in range(B):
            for ic in range(N // CHUNK):
                isl = slice(b * N + ic * CHUNK, b * N + (ic + 1) * CHUNK)
                if pending is not None:
                    y_state["y_sb"] = ypool.tile([MC, DT, CHUNK], f32, tag="y", name="y_sb")
                po0 = ps_o.tile([DH + 1, CHUNK], f32, tag="o0")
                po1 = ps_o.tile([DH + 1, CHUNK], f32, tag="o1")
                for jl in range(NJT_B):
                    jt = b * NJT_B + jl
                    jsl = slice(jt * JT, (jt + 1) * JT)
                    ps = ps_s.tile([128, 2, CHUNK], f32, tag="sy")
                    nc.tensor.matmul(
                        ps[:, 0, :], lhsT=kt_sb[0:64, jsl], rhs=pt_sb[0:64, isl],
                        start=True, stop=True, tile_position=(0, 0),
                    )
                    nc.tensor.matmul(
                        ps[:, 1, :], lhsT=kt_sb[64:128, jsl], rhs=pt_sb[64:128, isl],
                        start=True, stop=True, tile_position=(64, 0),
                    )
                    e = epool.tile([128, 2, CHUNK], f32r, tag="e")
                    nc.scalar.activation(e, ps, Exp, scale=C2)
                    nc.tensor.matmul(
                        po0, lhsT=v_sb[:, jt, 0 : DH + 1], rhs=e[:, 0, :],
                        start=(jl == 0), stop=(jl == NJT_B - 1),
                    )
                    nc.tensor.matmul(
                        po1, lhsT=v_sb[:, jt, DH + 1 : 2 * DH + 2], rhs=e[:, 1, :],
                        start=(jl == 0), stop=(jl == NJT_B - 1),
                    )
                    if pending is not None and jl < DT:
                        y_step(pending[0], jl)

                if pending is not None:
                    nc.sync.dma_start(
                        out=yt.rearrange("(a p) t -> p a t", p=128)[:, :, pending[1]],
                        in_=y_state["y_sb"],
                    )

                ohn = opool.tile([128, CHUNK], f32r, tag="ohn")
                for h, po in ((0, po0), (1, po1)):
                    rcp = tmp1.tile([1, CHUNK], f32, tag=f"rcp{h}")
                    nc.vector.reciprocal(rcp, po[DH : DH + 1, :])
                    bc = tmp1.tile([64, CHUNK], f32, tag=f"bc{h}")
                    nc.gpsimd.partition_broadcast(bc, rcp, channels=64)
                    nc.vector.tensor_mul(
                        ohn[h * 64 : (h + 1) * 64], po[0:DH, :], bc
                    )
                pending = (ohn, isl)

        y_state["y_sb"] = ypool.tile([MC, DT, CHUNK], f32, tag="y", name="y_sb")
        for ot in range(DT):
            y_step(pending[0], ot)
        nc.sync.dma_start(
            out=yt.rearrange("(a p) t -> p a t", p=128)[:, :, pending[1]],
            in_=y_state["y_sb"],
        )

    nc.compile()
    return nc


def kernel(x, w_q, w_k, w_v, w_o):
    from concourse.bass_utils import run_bass_kernel_spmd

    global last_results

    nc = _build()

    x = np.asarray(x, dtype=np.float32)
    w_q = np.asarray(w_q, dtype=np.float32)
    w_k = np.asarray(w_k, dtype=np.float32)
    w_v = np.asarray(w_v, dtype=np.float32)
    w_o = np.asarray(w_o, dtype=np.float32)

    xt = np.ascontiguousarray(x.reshape(NTOK, D_MODEL).T)
    wp_full = (C1 / C2) * w_q + w_k

    in_maps = []
    for c in range(NCORES):
        cols = slice(MC * c, MC * (c + 1))
        in_maps.append(
            {
                "xt": xt,
                "wp": np.ascontiguousarray(wp_full[:, cols]),
                "wk": np.ascontiguousarray(w_k[:, cols]),
                "wv": np.ascontiguousarray(w_v[:, cols]),
                "wo": np.ascontiguousarray(w_o[cols, :]),
            }
        )

    trace = bool(os.environ.get("KERNEL_TRACE"))
    last_results = run_bass_kernel_spmd(
        nc, in_maps, core_ids=list(range(NCORES)), trace=trace
    )
    acc = np.zeros((D_MODEL, NTOK), dtype=np.float64)
    for r in last_results.results:
        acc += r["yt"]
    return np.ascontiguousarray(acc.T).reshape(B, N, D_MODEL).astype(np.float32)



# revision 6
# speedup vs baseline: 1.1710x; 1.1710x over previous
"""Dual-metric (CE softmax) attention, head-sharded across 8 cores.

v2: restructured for engine overlap.
 - PSUM: 2 pools x 2 bufs x 2 banks (tag-shared by proj/scores/y and
   po/transpose/ksq-sum tiles) -> po double-buffered, no phase barrier.
 - One eb exp per chunk (batched [128,4,2]) instead of 8 tiny ones.
 - reciprocal_approx_fast for the softmax denominator (1 DVE op).
 - Norm/output chain of chunk c deferred into chunk c+1's jt loop.
"""

import os
from contextlib import ExitStack
from functools import lru_cache

import numpy as np

B = 2
N = 2048
D_MODEL = 1024
N_HEADS = 16
D_HEAD = 64
T_WAKE = 0.315
OMEGA_LANG = 1.0 - T_WAKE
OMEGA_GRAV = T_WAKE
C1 = OMEGA_LANG / np.sqrt(D_HEAD)
C2 = OMEGA_GRAV
CB = -OMEGA_GRAV / 2.0

NCORES = 8
MC = 2 * D_HEAD          # feature columns per core (2 heads)
NTOK = B * N
CHUNK = 512
NCHUNK = NTOK // CHUNK
JT = 128                 # j-tile (key block)
NJT = NTOK // JT         # 32 global j tiles
NJT_B = N // JT          # 16 j tiles per batch
SUBS = CHUNK // JT       # 4 j tiles per chunk
DT = D_MODEL // 128      # 8 contraction tiles for projections
DH = D_HEAD

last_results = None


@lru_cache(maxsize=1)
def _build():
    import concourse.bacc as bacc
    import concourse.mybir as mybir
    import concourse.tile as tile
    from concourse.masks import make_identity

    f32 = mybir.dt.float32
    f32r = mybir.dt.float32r
    Exp = mybir.ActivationFunctionType.Exp

    nc = bacc.Bacc(None, target_bir_lowering=False, debug=False)
    xt = nc.dram_tensor("xt", [D_MODEL, NTOK], f32r, kind="ExternalInput")
    wp = nc.dram_tensor("wp", [D_MODEL, MC], f32r, kind="ExternalInput")
    wk = nc.dram_tensor("wk", [D_MODEL, MC], f32r, kind="ExternalInput")
    wv = nc.dram_tensor("wv", [D_MODEL, MC], f32r, kind="ExternalInput")
    wo = nc.dram_tensor("wo", [MC, D_MODEL], f32r, kind="ExternalInput")
    yt = nc.dram_tensor("yt", [D_MODEL, NTOK], f32, kind="ExternalOutput")

    with ExitStack() as ctx:
        tc = ctx.enter_context(tile.TileContext(nc))

        const = ctx.enter_context(tc.tile_pool(name="const", bufs=1))
        persist = ctx.enter_context(tc.tile_pool(name="persist", bufs=1))
        xpool = ctx.enter_context(tc.tile_pool(name="xpool", bufs=2))
        tmp = ctx.enter_context(tc.tile_pool(name="tmp", bufs=2))
        epool = ctx.enter_context(tc.tile_pool(name="epool", bufs=4))
        opool = ctx.enter_context(tc.tile_pool(name="opool", bufs=2))
        ypool = ctx.enter_context(tc.tile_pool(name="ypool", bufs=2))
        # PSUM: exactly 8 banks. sy: scores/proj/y ([128,2,512] = 2 banks);
        # o: po accumulators / v transposes / ksq sums (2 banks).
        ps_s = ctx.enter_context(tc.tile_pool(name="ps_s", bufs=2, space="PSUM"))
        ps_o = ctx.enter_context(tc.tile_pool(name="ps_o", bufs=2, space="PSUM"))

        wp_sb = const.tile([128, DT, MC], f32r, tag="wp")
        wk_sb = const.tile([128, DT, MC], f32r, tag="wk")
        wv_sb = const.tile([128, DT, MC], f32r, tag="wv")
        wo_sb = const.tile([MC, DT, 128], f32r, tag="wo")
        nc.sync.dma_start(out=wp_sb, in_=wp.rearrange("(a p) m -> p a m", p=128))
        nc.sync.dma_start(out=wk_sb, in_=wk.rearrange("(a p) m -> p a m", p=128))
        nc.sync.dma_start(out=wv_sb, in_=wv.rearrange("(a p) m -> p a m", p=128))
        nc.sync.dma_start(out=wo_sb, in_=wo.rearrange("d (a o) -> d a o", o=128))

        ident = const.tile([128, 128], f32, tag="ident")
        make_identity(nc, ident)
        # cb2: block-diagonal CB so one matmul sums ksq per head:
        # rows 0:64 -> col 0 (head 0), rows 64:128 -> col 1 (head 1).
        cb2 = const.tile([128, 2], f32, tag="cb2")
        nc.vector.memset(cb2, 0.0)
        nc.vector.memset(cb2[0:64, 0:1], CB)
        nc.vector.memset(cb2[64:128, 1:2], CB)

        pt_sb = persist.tile([128, NTOK], f32r, tag="pt")
        kt_sb = persist.tile([128, NTOK], f32r, tag="kt")
        # v_sb[:, jt, h, 0:64] = eb-scaled v^T; [.., 64] = eb (norm row)
        v_sb = persist.tile([128, NJT, 2, DH + 1], f32r, tag="v")
        eb_sb = persist.tile([128, NJT, 2], f32, tag="eb")

        # ---------------- Phase 1: projections ----------------
        for t0 in range(NCHUNK):
            tsl = slice(t0 * CHUNK, (t0 + 1) * CHUNK)
            xt_sb = xpool.tile([128, DT, CHUNK], f32r, tag="xt")
            nc.sync.dma_start(
                out=xt_sb, in_=xt.rearrange("(a p) t -> p a t", p=128)[:, :, tsl]
            )

            pk = ps_s.tile([128, 2, CHUNK], f32, tag="sy", name="pk")
            for a in range(DT):
                nc.tensor.matmul(
                    pk[:, 0, :], lhsT=wk_sb[:, a, :], rhs=xt_sb[:, a, :],
                    start=(a == 0), stop=(a == DT - 1),
                )
            nc.vector.tensor_copy(kt_sb[:, tsl], pk[:, 0, :])
            ksq = tmp.tile([128, CHUNK], f32, tag="ksq")
            nc.scalar.square(ksq, pk[:, 0, :])

            psq = ps_o.tile([128, SUBS, 2], f32, tag="o", name="psq")
            for sub in range(SUBS):
                nc.tensor.matmul(
                    psq[:, sub, :],
                    lhsT=ksq[:, sub * JT : (sub + 1) * JT],
                    rhs=cb2,
                    start=True, stop=True,
                )
            nc.scalar.activation(
                eb_sb[:, t0 * SUBS : (t0 + 1) * SUBS, :], psq, Exp
            )

            pp = ps_s.tile([128, 2, CHUNK], f32, tag="sy", name="pp")
            for a in range(DT):
                nc.tensor.matmul(
                    pp[:, 0, :], lhsT=wp_sb[:, a, :], rhs=xt_sb[:, a, :],
                    start=(a == 0), stop=(a == DT - 1),
                )
            nc.vector.tensor_copy(pt_sb[:, tsl], pp[:, 0, :])

            pv = ps_s.tile([128, 2, CHUNK], f32, tag="sy", name="pv")
            for a in range(DT):
                nc.tensor.matmul(
                    pv[:, 0, :], lhsT=wv_sb[:, a, :], rhs=xt_sb[:, a, :],
                    start=(a == 0), stop=(a == DT - 1),
                )
            vt = tmp.tile([128, CHUNK], f32, tag="vt")
            nc.vector.tensor_copy(vt, pv[:, 0, :])

            for sub in range(SUBS):
                jt = t0 * SUBS + sub
                ptr = ps_o.tile([128, 2, CHUNK], f32, tag="o", name="ptr")
                nc.tensor.transpose(
                    ptr[:, 0, 0:128], vt[:, sub * JT : (sub + 1) * JT], ident
                )
                for h in range(2):
                    nc.vector.tensor_scalar_mul(
                        v_sb[:, jt, h, 0:DH],
                        ptr[:, 0, h * 64 : h * 64 + 64],
                        eb_sb[:, jt, h : h + 1],
                    )
                    nc.vector.tensor_copy(
                        v_sb[:, jt, h, DH : DH + 1], eb_sb[:, jt, h : h + 1]
                    )

        # ---------------- Phase 2: attention ----------------
        def emit_pending(pend, jl):
            """Deferred norm/output chain of the previous chunk, spread
            across the current chunk's jt steps."""
            po, isl, st = pend["po"], pend["isl"], pend["state"]
            if jl == 0:
                nrm = tmp.tile([1, 2, CHUNK], f32, tag="nrm", name="nrm")
                nc.vector.tensor_copy(nrm, po[DH : DH + 1, :, :])
                rcp = tmp.tile([1, 2, CHUNK], f32, tag="rcp", name="rcp")
                nc.vector.reciprocal_approx_fast(rcp, nrm)
                st["rcp"] = rcp
            elif jl == 1:
                bc0 = tmp.tile([64, CHUNK], f32, tag="bc0", name="bc0")
                nc.gpsimd.partition_broadcast(bc0, st["rcp"][:, 0, :], channels=64)
                st["bc0"] = bc0
            elif jl == 2:
                bc1 = tmp.tile([64, CHUNK], f32, tag="bc1", name="bc1")
                nc.gpsimd.partition_broadcast(bc1, st["rcp"][:, 1, :], channels=64)
                st["bc1"] = bc1
            elif jl == 3:
                ohn = opool.tile([MC, CHUNK], f32r, tag="ohn", name="ohn")
                nc.vector.tensor_mul(ohn[0:64, :], po[0:DH, 0, :], st["bc0"])
                st["ohn"] = ohn
            elif jl == 4:
                nc.vector.tensor_mul(
                    st["ohn"][64:128, :], po[0:DH, 1, :], st["bc1"]
                )
                st["y_sb"] = ypool.tile([MC, DT, CHUNK], f32, tag="y", name="y_sb")
            elif 5 <= jl < 5 + DT:
                ot = jl - 5
                py = ps_s.tile([128, 2, CHUNK], f32, tag="sy", name="py")
                nc.tensor.matmul(
                    py[:, 0, :], lhsT=wo_sb[:, ot, :], rhs=st["ohn"],
                    start=True, stop=True,
                )
                nc.vector.tensor_copy(st["y_sb"][:, ot, :], py[:, 0, :])
                if ot == DT - 1:
                    nc.sync.dma_start(
                        out=yt.rearrange("(a p) t -> p a t", p=128)[:, :, isl],
                        in_=st["y_sb"],
                    )

        pending = None
        for b in range(B):
            for ic in range(N // CHUNK):
                isl = slice(b * N + ic * CHUNK, b * N + (ic + 1) * CHUNK)
                po = ps_o.tile([DH + 1, 2, CHUNK], f32, tag="o", name="po")
                for jl in range(NJT_B):
                    jt = b * NJT_B + jl
                    jsl = slice(jt * JT, (jt + 1) * JT)
                    ps = ps_s.tile([128, 2, CHUNK], f32, tag="sy", name="ps")
                    nc.tensor.matmul(
                        ps[:, 0, :], lhsT=kt_sb[0:64, jsl], rhs=pt_sb[0:64, isl],
                        start=True, stop=True, tile_position=(0, 0),
                    )
                    nc.tensor.matmul(
                        ps[:, 1, :], lhsT=kt_sb[64:128, jsl], rhs=pt_sb[64:128, isl],
                        start=True, stop=True, tile_position=(64, 0),
                    )
                    e = epool.tile([128, 2, CHUNK], f32r, tag="e")
                    nc.scalar.activation(e, ps, Exp, scale=C2)
                    nc.tensor.matmul(
                        po[:, 0, :], lhsT=v_sb[:, jt, 0, :], rhs=e[:, 0, :],
                        start=(jl == 0), stop=(jl == NJT_B - 1),
                    )
                    nc.tensor.matmul(
                        po[:, 1, :], lhsT=v_sb[:, jt, 1, :], rhs=e[:, 1, :],
                        start=(jl == 0), stop=(jl == NJT_B - 1),
                    )
                    if pending is not None:
                        emit_pending(pending, jl)
                pending = {"po": po, "isl": isl, "state": {}}

        for jl in range(NJT_B):
            emit_pending(pending, jl)

    nc.compile()
    return nc


def kernel(x, w_q, w_k, w_v, w_o):
    from concourse.bass_utils import run_bass_kernel_spmd

    global last_results

    nc = _build()

    x = np.asarray(x, dtype=np.float32)
    w_q = np.asarray(w_q, dtype=np.float32)
    w_k = np.asarray(w_k, dtype=np.float32)
    w_v = np.asarray(w_v, dtype=np.float32)
    w_o = np.asarray(w_o, dtype=np.float32)

    xt = np.ascontiguousarray(x.reshape(NTOK, D_MODEL).T)
    wp_full = (C1 / C2) * w_q + w_k

    in_maps = []
    for c in range(NCORES):
        cols = slice(MC * c, MC * (c + 1))
        in_maps.append(
            {
                "xt": xt,
                "wp": np.ascontiguousarray(wp_full[:, cols]),
                "wk": np.ascontiguousarray(w_k[:, cols]),
                "wv": np.ascontiguousarray(w_v[:, cols]),
                "wo": np.ascontiguousarray(w_o[cols, :]),
            }
        )

    trace = bool(os.environ.get("KERNEL_TRACE"))
    last_results = run_bass_kernel_spmd(
        nc, in_maps, core_ids=list(range(NCORES)), trace=trace
    )
    acc = np.zeros((D_MODEL, NTOK), dtype=np.float64)
    for r in last_results.results:
        acc += r["yt"]
    return np.ascontiguousarray(acc.T).reshape(B, N, D_MODEL).astype(np.float32)


# revision 9
# speedup vs baseline: 1.6818x; 1.4362x over previous
"""Dual-metric (CE softmax) attention, head-sharded across 8 cores.

v2: restructured for engine overlap.
 - PSUM: 2 pools x 2 bufs x 2 banks (tag-shared by proj/scores/y and
   po/transpose/ksq-sum tiles) -> po double-buffered, no phase barrier.
 - One eb exp per chunk (batched [128,4,2]) instead of 8 tiny ones.
 - reciprocal_approx_fast for the softmax denominator (1 DVE op).
 - Norm/output chain of chunk c deferred into chunk c+1's jt loop.
"""

import os
from contextlib import ExitStack
from functools import lru_cache

import numpy as np

B = 2
N = 2048
D_MODEL = 1024
N_HEADS = 16
D_HEAD = 64
T_WAKE = 0.315
OMEGA_LANG = 1.0 - T_WAKE
OMEGA_GRAV = T_WAKE
C1 = OMEGA_LANG / np.sqrt(D_HEAD)
C2 = OMEGA_GRAV
CB = -OMEGA_GRAV / 2.0

NCORES = 8
MC = 2 * D_HEAD          # feature columns per core (2 heads)
NTOK = B * N
CHUNK = 512
NCHUNK = NTOK // CHUNK
JT = 128                 # j-tile (key block)
NJT = NTOK // JT         # 32 global j tiles
NJT_B = N // JT          # 16 j tiles per batch
SUBS = CHUNK // JT       # 4 j tiles per chunk
DT = D_MODEL // 128      # 8 contraction tiles for projections
DH = D_HEAD

last_results = None


@lru_cache(maxsize=1)
def _build():
    import concourse.bacc as bacc
    import concourse.mybir as mybir
    import concourse.tile as tile
    from concourse.masks import make_identity

    f32 = mybir.dt.float32
    f32r = mybir.dt.float32r
    Exp = mybir.ActivationFunctionType.Exp

    nc = bacc.Bacc(None, target_bir_lowering=False, debug=False)
    xt = nc.dram_tensor("xt", [D_MODEL, NTOK], f32r, kind="ExternalInput")
    wp = nc.dram_tensor("wp", [D_MODEL, MC], f32r, kind="ExternalInput")
    wk = nc.dram_tensor("wk", [D_MODEL, MC], f32r, kind="ExternalInput")
    wv = nc.dram_tensor("wv", [D_MODEL, MC], f32r, kind="ExternalInput")
    wo = nc.dram_tensor("wo", [MC, D_MODEL], f32r, kind="ExternalInput")
    yt = nc.dram_tensor("yt", [D_MODEL, NTOK], f32, kind="ExternalOutput")

    with ExitStack() as ctx:
        tc = ctx.enter_context(tile.TileContext(nc))

        const = ctx.enter_context(tc.tile_pool(name="const", bufs=1))
        persist = ctx.enter_context(tc.tile_pool(name="persist", bufs=1))
        xpool = ctx.enter_context(tc.tile_pool(name="xpool", bufs=2))
        tmp = ctx.enter_context(tc.tile_pool(name="tmp", bufs=2))
        epool = ctx.enter_context(tc.tile_pool(name="epool", bufs=4))
        opool = ctx.enter_context(tc.tile_pool(name="opool", bufs=2))
        ypool = ctx.enter_context(tc.tile_pool(name="ypool", bufs=2))
        # PSUM: exactly 8 banks. sy: scores/proj/y ([128,2,512] = 2 banks);
        # o: po accumulators / v transposes / ksq sums (2 banks).
        ps_s = ctx.enter_context(tc.tile_pool(name="ps_s", bufs=2, space="PSUM"))
        ps_o = ctx.enter_context(tc.tile_pool(name="ps_o", bufs=2, space="PSUM"))

        wp_sb = const.tile([128, DT, MC], f32r, tag="wp")
        wk_sb = const.tile([128, DT, MC], f32r, tag="wk")
        wv_sb = const.tile([128, DT, MC], f32r, tag="wv")
        wo_sb = const.tile([MC, DT, 128], f32r, tag="wo")
        nc.sync.dma_start(out=wp_sb, in_=wp.rearrange("(a p) m -> p a m", p=128))
        nc.sync.dma_start(out=wk_sb, in_=wk.rearrange("(a p) m -> p a m", p=128))
        nc.sync.dma_start(out=wv_sb, in_=wv.rearrange("(a p) m -> p a m", p=128))
        nc.sync.dma_start(out=wo_sb, in_=wo.rearrange("d (a o) -> d a o", o=128))

        ident = const.tile([128, 128], f32, tag="ident")
        make_identity(nc, ident)
        # cb2: block-diagonal CB so one matmul sums ksq per head:
        # rows 0:64 -> col 0 (head 0), rows 64:128 -> col 1 (head 1).
        cb2 = const.tile([128, 2], f32, tag="cb2")
        nc.vector.memset(cb2, 0.0)
        nc.vector.memset(cb2[0:64, 0:1], CB)
        nc.vector.memset(cb2[64:128, 1:2], CB)

        pt_sb = persist.tile([128, NTOK], f32r, tag="pt")
        kt_sb = persist.tile([128, NTOK], f32r, tag="kt")
        # v_sb[:, jt, h, 0:64] = eb-scaled v^T; [.., 64] = eb (norm row)
        v_sb = persist.tile([128, NJT, 2, DH + 1], f32r, tag="v")
        eb_sb = persist.tile([128, NJT, 2], f32, tag="eb")

        # ---------------- Phase 1: projections ----------------
        for t0 in range(NCHUNK):
            tsl = slice(t0 * CHUNK, (t0 + 1) * CHUNK)
            xt_sb = xpool.tile([128, DT, CHUNK], f32r, tag="xt")
            nc.sync.dma_start(
                out=xt_sb, in_=xt.rearrange("(a p) t -> p a t", p=128)[:, :, tsl]
            )

            pk = ps_s.tile([128, 2, CHUNK], f32, tag="sy", name="pk")
            for a in range(DT):
                nc.tensor.matmul(
                    pk[:, 0, :], lhsT=wk_sb[:, a, :], rhs=xt_sb[:, a, :],
                    start=(a == 0), stop=(a == DT - 1),
                )
            nc.vector.tensor_copy(kt_sb[:, tsl], pk[:, 0, :])
            ksq = tmp.tile([128, CHUNK], f32, tag="ksq")
            nc.scalar.square(ksq, pk[:, 0, :])

            pp = ps_s.tile([128, 2, CHUNK], f32, tag="sy", name="pp")
            for a in range(DT):
                nc.tensor.matmul(
                    pp[:, 0, :], lhsT=wp_sb[:, a, :], rhs=xt_sb[:, a, :],
                    start=(a == 0), stop=(a == DT - 1),
                )
            nc.vector.tensor_copy(pt_sb[:, tsl], pp[:, 0, :])

            pv = ps_s.tile([128, 2, CHUNK], f32, tag="sy", name="pv")
            for a in range(DT):
                nc.tensor.matmul(
                    pv[:, 0, :], lhsT=wv_sb[:, a, :], rhs=xt_sb[:, a, :],
                    start=(a == 0), stop=(a == DT - 1),
                )
            vt = tmp.tile([128, CHUNK], f32, tag="vt")
            nc.vector.tensor_copy(vt, pv[:, 0, :])

            psq = ps_o.tile([128, SUBS, 2], f32, tag="o", name="psq")
            for sub in range(SUBS):
                nc.tensor.matmul(
                    psq[:, sub, :],
                    lhsT=ksq[:, sub * JT : (sub + 1) * JT],
                    rhs=cb2,
                    start=True, stop=True,
                )
            nc.scalar.activation(
                eb_sb[:, t0 * SUBS : (t0 + 1) * SUBS, :], psq, Exp
            )

            for sub in range(SUBS):
                jt = t0 * SUBS + sub
                ptr = ps_o.tile([128, 2, CHUNK], f32, tag="o", name="ptr")
                nc.tensor.transpose(
                    ptr[:, 0, 0:128], vt[:, sub * JT : (sub + 1) * JT], ident
                )
                for h in range(2):
                    nc.vector.tensor_scalar_mul(
                        v_sb[:, jt, h, 0:DH],
                        ptr[:, 0, h * 64 : h * 64 + 64],
                        eb_sb[:, jt, h : h + 1],
                    )
                    nc.vector.tensor_copy(
                        v_sb[:, jt, h, DH : DH + 1], eb_sb[:, jt, h : h + 1]
                    )

        # ---------------- Phase 2: attention ----------------
        def emit_pending(pend, jl):
            """Deferred norm/output chain of the previous chunk, spread
            across the current chunk's jt steps."""
            po, isl, st = pend["po"], pend["isl"], pend["state"]
            if jl == 0:
                nrm = tmp.tile([1, 2, CHUNK], f32, tag="nrm", name="nrm")
                nc.vector.tensor_copy(nrm, po[DH : DH + 1, :, :])
                rcp = tmp.tile([1, 2, CHUNK], f32, tag="rcp", name="rcp")
                nc.vector.reciprocal_approx_fast(rcp, nrm)
                st["rcp"] = rcp
            elif jl == 1:
                bc0 = tmp.tile([64, CHUNK], f32, tag="bc0", name="bc0")
                nc.gpsimd.partition_broadcast(bc0, st["rcp"][:, 0, :], channels=64)
                st["bc0"] = bc0
            elif jl == 2:
                bc1 = tmp.tile([64, CHUNK], f32, tag="bc1", name="bc1")
                nc.gpsimd.partition_broadcast(bc1, st["rcp"][:, 1, :], channels=64)
                st["bc1"] = bc1
            elif jl == 3:
                ohn = opool.tile([MC, CHUNK], f32r, tag="ohn", name="ohn")
                nc.vector.tensor_mul(ohn[0:64, :], po[0:DH, 0, :], st["bc0"])
                st["ohn"] = ohn
            elif jl == 4:
                nc.vector.tensor_mul(
                    st["ohn"][64:128, :], po[0:DH, 1, :], st["bc1"]
                )
                st["y_sb"] = ypool.tile([MC, DT, CHUNK], f32, tag="y", name="y_sb")
            elif 5 <= jl < 5 + DT:
                ot = jl - 5
                py = ps_s.tile([128, 2, CHUNK], f32, tag="sy", name="py")
                nc.tensor.matmul(
                    py[:, 0, :], lhsT=wo_sb[:, ot, :], rhs=st["ohn"],
                    start=True, stop=True,
                )
                nc.vector.tensor_copy(st["y_sb"][:, ot, :], py[:, 0, :])
                if ot == DT - 1:
                    nc.sync.dma_start(
                        out=yt.rearrange("(a p) t -> p a t", p=128)[:, :, isl],
                        in_=st["y_sb"],
                    )

        def emit_scores(isl, jt):
            jsl = slice(jt * JT, (jt + 1) * JT)
            ps = ps_s.tile([128, 2, CHUNK], f32, tag="sy", name="ps")
            nc.tensor.matmul(
                ps[:, 0, :], lhsT=kt_sb[0:64, jsl], rhs=pt_sb[0:64, isl],
                start=True, stop=True, tile_position=(0, 0),
            )
            nc.tensor.matmul(
                ps[:, 1, :], lhsT=kt_sb[64:128, jsl], rhs=pt_sb[64:128, isl],
                start=True, stop=True, tile_position=(64, 0),
            )
            return ps

        pending = None
        for b in range(B):
            for ic in range(N // CHUNK):
                isl = slice(b * N + ic * CHUNK, b * N + (ic + 1) * CHUNK)
                po = ps_o.tile([DH + 1, 2, CHUNK], f32, tag="o", name="po")
                ps_cur = emit_scores(isl, b * NJT_B)
                for jl in range(NJT_B):
                    jt = b * NJT_B + jl
                    # next jt's scores go ahead of this jt's AV on the PE
                    # queue so the PE streams through exp latency.
                    ps_nxt = (
                        emit_scores(isl, jt + 1) if jl + 1 < NJT_B else None
                    )
                    e = epool.tile([128, 2, CHUNK], f32r, tag="e")
                    nc.scalar.activation(e, ps_cur, Exp, scale=C2)
                    if pending is not None:
                        emit_pending(pending, jl)
                    nc.tensor.matmul(
                        po[:, 0, :], lhsT=v_sb[:, jt, 0, :], rhs=e[:, 0, :],
                        start=(jl == 0), stop=(jl == NJT_B - 1),
                    )
                    nc.tensor.matmul(
                        po[:, 1, :], lhsT=v_sb[:, jt, 1, :], rhs=e[:, 1, :],
                        start=(jl == 0), stop=(jl == NJT_B - 1),
                    )
                    ps_cur = ps_nxt
                pending = {"po": po, "isl": isl, "state": {}}

        for jl in range(NJT_B):
            emit_pending(pending, jl)

    nc.compile()
    return nc


def kernel(x, w_q, w_k, w_v, w_o):
    from concourse.bass_utils import run_bass_kernel_spmd

    global last_results

    nc = _build()

    x = np.asarray(x, dtype=np.float32)
    w_q = np.asarray(w_q, dtype=np.float32)
    w_k = np.asarray(w_k, dtype=np.float32)
    w_v = np.asarray(w_v, dtype=np.float32)
    w_o = np.asarray(w_o, dtype=np.float32)

    xt = np.ascontiguousarray(x.reshape(NTOK, D_MODEL).T)
    wp_full = (C1 / C2) * w_q + w_k

    in_maps = []
    for c in range(NCORES):
        cols = slice(MC * c, MC * (c + 1))
        in_maps.append(
            {
                "xt": xt,
                "wp": np.ascontiguousarray(wp_full[:, cols]),
                "wk": np.ascontiguousarray(w_k[:, cols]),
                "wv": np.ascontiguousarray(w_v[:, cols]),
                "wo": np.ascontiguousarray(w_o[cols, :]),
            }
        )

    trace = bool(os.environ.get("KERNEL_TRACE"))
    last_results = run_bass_kernel_spmd(
        nc, in_maps, core_ids=list(range(NCORES)), trace=trace
    )
    acc = np.zeros((D_MODEL, NTOK), dtype=np.float64)
    for r in last_results.results:
        acc += r["yt"]
    return np.ascontiguousarray(acc.T).reshape(B, N, D_MODEL).astype(np.float32)
